# revision 1
# baseline (speedup 1.0000x reference)
"""DCell-style hierarchical NN (gather -> 3x [Linear+Tanh+BatchNorm] -> root)
on 8 Trainium2 NeuronCores.

Sharding: tree-sharding ("model parallel over subsystems") — core c owns
L1 subsystems [64c, 64c+64), L2 parents [8c, 8c+8), L3 parent c, and the
FULL batch B=4096.  All BatchNorm statistics at levels 1-3 are then
core-local (full batch present on every core); the only cross-core
communication is a single 1 MB AllReduce of the root layer's partial
products.  BatchNorm is folded at runtime into the next level's weights:
with h = tanh(W x + b) kept pre-BN, o = a*h + c where
a = g*rsqrt(var+eps), c = beta - mean*a, so the next level uses
W' = W*a (per input column) and b' = b + W@c.

Device layout: features on partitions, batch on the free axis.  Matmul
operands are bf16 (PE runs 1 cycle/row vs 4 for fp32), accumulation /
statistics / normalization all fp32.  L1 uses block-diagonal weights
(8 subsystems of 16->20 per K=128 matmul).  L2 outputs are packed four
parents per PSUM bank in 32-row slots (24 real features zero-padded to
32 — PE column tiles must be 32-aligned).  Stats use bn_stats/bn_aggr
(one DVE pass for mean+var); rsqrt is a Newton iteration on the DVE to
avoid switching the ACT table set away from Tanh.  Matmuls of the same
PE tiling mode are batched to avoid mode-switch drains.
"""

import numpy as np
import ml_dtypes

BF16 = ml_dtypes.bfloat16
N_CORES = 8
B = 4096
BT = 512
NBT = B // BT
EPS = 1e-5
MAGIC = 0x5F3759DF

_PROG = None  # cached program across calls


# ---------------------------------------------------------------- device code

def _rsqrt_newton(nc, AL, y, s, t, magic, iters=3):
    """y = rsqrt(s), all APs same shape, f32 (magic: int32)."""
    import concourse.mybir as mybir
    i32 = mybir.dt.int32
    nc.vector.tensor_scalar(out=t.bitcast(i32), in0=s.bitcast(i32),
                            scalar1=1, scalar2=None, op0=AL.arith_shift_right)
    nc.vector.tensor_tensor(out=y.bitcast(i32), in0=magic, in1=t.bitcast(i32),
                            op=AL.subtract)
    for _ in range(iters):
        nc.vector.tensor_tensor(out=t, in0=y, in1=y, op=AL.mult)
        nc.vector.tensor_tensor(out=t, in0=t, in1=s, op=AL.mult)
        nc.vector.tensor_scalar(out=t, in0=t, scalar1=-0.5, scalar2=1.5,
                                op0=AL.mult, op1=AL.add)
        nc.vector.tensor_tensor(out=y, in0=y, in1=t, op=AL.mult)


def build_program():
    import concourse.bacc as bacc
    import concourse.mybir as mybir
    import concourse.tile as tile

    f32 = mybir.dt.float32
    bf16 = mybir.dt.bfloat16
    i32 = mybir.dt.int32
    AL = mybir.AluOpType
    TANH = mybir.ActivationFunctionType.Tanh
    IDENT = mybir.ActivationFunctionType.Identity

    nc = bacc.Bacc("TRN2", target_bir_lowering=False, debug=False,
                   num_devices=N_CORES)

    # ------------------------------------------------ DRAM I/O (per core)
    xgd = nc.dram_tensor("xg", [8, 128, B], bf16, kind="ExternalInput")
    w1d = nc.dram_tensor("w1", [128, 1280], bf16, kind="ExternalInput")
    w2ad = nc.dram_tensor("w2a", [128, 256], f32, kind="ExternalInput")
    w2bd = nc.dram_tensor("w2b", [128, 256], f32, kind="ExternalInput")
    sad = nc.dram_tensor("sa", [128, 30], f32, kind="ExternalInput")
    s96d = nc.dram_tensor("s96", [128, 68], f32, kind="ExternalInput")
    s32d = nc.dram_tensor("s32", [32, 67], f32, kind="ExternalInput")
    s64d = nc.dram_tensor("s64", [64, 67], f32, kind="ExternalInput")
    b2hd = nc.dram_tensor("b2h", [128, 2], f32, kind="ExternalInput")
    outd = nc.dram_tensor("out", [B // N_CORES, 64], f32, kind="ExternalOutput")
    cc_in = nc.dram_tensor("cc_in", [64, B], bf16)
    cc_out = nc.dram_tensor("cc_out", [64, B], bf16, addr_space="Shared")

    with tile.TileContext(nc) as tc:
        sbS = tc.alloc_tile_pool(name="sbS", bufs=1)
        sbH1 = tc.alloc_tile_pool(name="sbH1", bufs=1)
        sbX = tc.alloc_tile_pool(name="sbX", bufs=1, side="right")
        psmm = tc.alloc_tile_pool(name="psmm", bufs=4, space="PSUM")

        # static tiles
        w1sb = sbS.tile([128, 1280], bf16, name="w1sb")
        w2asb = sbS.tile([128, 256], f32, name="w2asb")
        w2bsb = sbS.tile([128, 256], f32, name="w2bsb")
        sAsb = sbS.tile([128, 30], f32, name="sAsb")
        s96sb = sbS.tile([128, 68], f32, name="s96sb")
        s32sb = sbS.tile([32, 67], f32, name="s32sb")
        s64sb = sbS.tile([64, 67], f32, name="s64sb")
        b2hsb = sbS.tile([128, 2], f32, name="b2hsb")
        stA = sbS.tile([128, 384], f32, name="stA")   # 8 parents * 8bt * 6
        stB = sbS.tile([128, 96], f32, name="stB")    # 2 groups
        st2 = sbS.tile([128, 96], f32, name="st2")
        st3 = sbS.tile([32, 48], f32, name="st3")
        stR = sbS.tile([64, 48], f32, name="stR")
        aggA = sbS.tile([128, 16], f32, name="aggA")
        aggB = sbS.tile([128, 4], f32, name="aggB")
        agg2 = sbS.tile([128, 4], f32, name="agg2")
        agg3 = sbS.tile([32, 2], f32, name="agg3")
        aggR = sbS.tile([64, 2], f32, name="aggR")
        magic = sbS.tile([128, 16], i32, name="magic")
        nsS = sbS.tile([128, 16], f32, name="nsS")
        nsT = sbS.tile([128, 16], f32, name="nsT")
        nsY = sbS.tile([128, 16], f32, name="nsY")
        ctm = sbS.tile([128, 8], f32, name="ctm")
        aA = sbS.tile([128, 8], f32, name="aA")
        cA = sbS.tile([128, 8], f32, name="cA")
        aB = sbS.tile([128, 2], f32, name="aB")
        cB = sbS.tile([128, 2], f32, name="cB")
        a2 = sbS.tile([128, 2], f32, name="a2")
        c2 = sbS.tile([128, 2], f32, name="c2")
        a3 = sbS.tile([32, 1], f32, name="a3")
        c3 = sbS.tile([32, 1], f32, name="c3")
        srt = sbS.tile([64, 1], f32, name="srt")
        drt = sbS.tile([64, 1], f32, name="drt")
        w2abf = sbS.tile([128, 256], bf16, name="w2abf")
        w2bbf = sbS.tile([128, 256], bf16, name="w2bbf")
        w3abf = sbS.tile([128, 32], bf16, name="w3abf")
        w3bbf = sbS.tile([128, 32], bf16, name="w3bbf")
        wrbf = sbS.tile([32, 64], bf16, name="wrbf")
        b2p = sbS.tile([128, 2], f32, name="b2p")
        b3sb = sbS.tile([32, 1], f32, name="b3sb")
        brc = sbS.tile([64, 1], f32, name="brc")

        h1a = sbH1.tile([128, 8 * B], bf16, name="h1a")
        h1b = sbH1.tile([128, 2 * B], bf16, name="h1b")
        xsb = sbX.tile([128, 8 * B], bf16, name="xsb")

        nc.vector.memset(magic[:], MAGIC)

        # ------------------------------------------------ input DMAs
        # first weight + first X chunks first so compute starts ASAP
        nc.sync.dma_start(w1sb[:], w1d[:])
        for p in range(8):
            nc.sync.dma_start(xsb[:, p * B:(p + 1) * B], xgd[p, :, :])
        nc.sync.dma_start(w2asb[:], w2ad[:])
        nc.sync.dma_start(w2bsb[:], w2bd[:])
        nc.sync.dma_start(sAsb[:], sad[:])
        nc.sync.dma_start(s96sb[:], s96d[:])
        nc.sync.dma_start(s32sb[:], s32d[:])
        nc.sync.dma_start(s64sb[:], s64d[:])
        nc.sync.dma_start(b2hsb[:], b2hd[:])

        # ------------------------------------------------ level 1
        # A pass: per-parent [K=128 genes] -> [128 features] (mode 128x128),
        # weight-stationary (p outer), batch-tile pairs share a 2-bank psum
        # so each ACT covers [128, 1024].  B pass (remaining 32 features x 4
        # parents packed per bank, mode 128x32) is interleaved per group so
        # its stats finish early.
        def l1_a(p):
            for b2 in range(NBT // 2):
                c0 = b2 * 2 * BT
                psA = psmm.tile([128, 2 * BT], f32, name=f"psA_{p}_{b2}",
                                tag="mm")
                for h in range(2):
                    nc.tensor.matmul(
                        psA[:, h * BT:(h + 1) * BT],
                        w1sb[:, p * 160: p * 160 + 128],
                        xsb[:, p * B + c0 + h * BT: p * B + c0 + (h + 1) * BT],
                        start=True, stop=True)
                ha = h1a[:, p * B + c0: p * B + c0 + 2 * BT]
                nc.scalar.activation(ha, psA[:], TANH, bias=sAsb[:, p:p + 1])
                for h in range(2):
                    bt = 2 * b2 + h
                    nc.vector.bn_stats(
                        stA[:, p * 48 + bt * 6: p * 48 + bt * 6 + 6],
                        h1a[:, p * B + bt * BT: p * B + (bt + 1) * BT])
            nc.vector.bn_aggr(aggA[:, 2 * p:2 * p + 2],
                              stA[:, p * 48:(p + 1) * 48])

        def l1_b(g, b2s):
            for b2 in b2s:
                c0 = b2 * 2 * BT
                psB = psmm.tile([128, 2 * BT], f32, name=f"psB_{b2}_{g}",
                                tag="mm")
                for h in range(2):
                    ch = c0 + h * BT
                    for q in range(4):
                        p = 4 * g + q
                        nc.tensor.matmul(
                            psB[32 * q:32 * q + 32, h * BT:(h + 1) * BT],
                            w1sb[:, p * 160 + 128:(p + 1) * 160],
                            xsb[:, p * B + ch: p * B + ch + BT],
                            start=True, stop=True,
                            tile_position=(0, 32 * q),
                            skip_group_check=True)
                hb = h1b[:, g * B + c0: g * B + c0 + 2 * BT]
                nc.scalar.activation(hb, psB[:], TANH,
                                     bias=sAsb[:, 24 + g:25 + g])
                for h in range(2):
                    bt = 2 * b2 + h
                    nc.vector.bn_stats(
                        stB[:, g * 48 + bt * 6: g * 48 + bt * 6 + 6],
                        h1b[:, g * B + bt * BT: g * B + (bt + 1) * BT])

        for p in range(4):
            l1_a(p)
        l1_b(0, range(0, 2))
        l1_a(4)
        l1_b(0, range(2, 4))
        nc.vector.bn_aggr(aggB[:, 0:2], stB[:, 0:48])
        for p in range(5, 8):
            l1_a(p)
        l1_b(1, range(0, 4))
        nc.vector.bn_aggr(aggB[:, 2:4], stB[:, 48:96])
        sbX.release()

        # ------------------------------------------------ L1 stats -> fold
        nc.vector.tensor_scalar(out=nsS[:, 0:8], in0=aggA[:, 1::2],
                                scalar1=EPS, scalar2=None, op0=AL.add)
        _rsqrt_newton(nc, AL, nsY[:, 0:8], nsS[:, 0:8], nsT[:, 0:8], magic[:, 0:8])
        nc.vector.tensor_tensor(out=aA[:], in0=nsY[:, 0:8], in1=sAsb[:, 8:16],
                                op=AL.mult)
        nc.vector.tensor_tensor(out=ctm[:], in0=aggA[:, 0::2], in1=aA[:], op=AL.mult)
        nc.vector.tensor_tensor(out=cA[:], in0=sAsb[:, 16:24], in1=ctm[:],
                                op=AL.subtract)
        nc.vector.tensor_scalar(out=nsS[:, 8:10], in0=aggB[:, 1::2],
                                scalar1=EPS, scalar2=None, op0=AL.add)
        _rsqrt_newton(nc, AL, nsY[:, 8:10], nsS[:, 8:10], nsT[:, 8:10], magic[:, 8:10])
        nc.vector.tensor_tensor(out=aB[:], in0=nsY[:, 8:10], in1=sAsb[:, 26:28],
                                op=AL.mult)
        nc.vector.tensor_tensor(out=ctm[:, 0:2], in0=aggB[:, 0::2], in1=aB[:],
                                op=AL.mult)
        nc.vector.tensor_tensor(out=cB[:], in0=sAsb[:, 28:30], in1=ctm[:, 0:2],
                                op=AL.subtract)
        # fold BN1 into W2 (bf16) and bias.  w2b is block-diagonal per group
        # ([128, 128] covering 4 parents), so its fold is one op per group.
        for p in range(8):
            nc.vector.tensor_scalar(out=w2abf[:, 32 * p:32 * p + 32],
                                    in0=w2asb[:, 32 * p:32 * p + 32],
                                    scalar1=aA[:, p:p + 1], scalar2=None,
                                    op0=AL.mult)
        for g in range(2):
            nc.vector.tensor_scalar(out=w2bbf[:, 128 * g:128 * g + 128],
                                    in0=w2bsb[:, 128 * g:128 * g + 128],
                                    scalar1=aB[:, g:g + 1], scalar2=None,
                                    op0=AL.mult)
        psT2 = [psmm.tile([128, BT], f32, name=f"psb2_{g}", tag="mm")
                for g in range(2)]
        for g in range(2):
            for q in range(4):
                p = 4 * g + q
                nc.tensor.matmul(psT2[g][32 * q:32 * q + 32, 0:1],
                                 w2asb[:, 32 * p:32 * p + 32], cA[:, p:p + 1],
                                 start=True, stop=False,
                                 tile_position=(0, 32 * q),
                                 skip_group_check=True)
        for g in range(2):
            nc.tensor.matmul(psT2[g][:, 0:1], w2bsb[:, 128 * g:128 * g + 128],
                             cB[:, g:g + 1], start=False, stop=True,
                             skip_group_check=True)
        for g in range(2):
            nc.scalar.activation(b2p[:, g:g + 1], psT2[g][:, 0:1], IDENT,
                                 bias=b2hsb[:, g:g + 1])

        # ------------------------------------------------ level 2
        sbH2 = tc.alloc_tile_pool(name="sbH2", bufs=1, side="right")
        h2 = sbH2.tile([128, 2 * B], bf16, name="h2")
        for b2 in range(NBT // 2):
            c0 = b2 * 2 * BT
            ps2g = [psmm.tile([128, 2 * BT], f32, name=f"ps2_{b2}_{g}",
                              tag="mm") for g in range(2)]
            # A contributions: (128x32) col-tiled mode; both batch halves
            # back-to-back so each stationary weight is loaded once
            for g in range(2):
                for q in range(4):
                    p = 4 * g + q
                    for h in range(2):
                        ch = c0 + h * BT
                        nc.tensor.matmul(
                            ps2g[g][32 * q:32 * q + 32, h * BT:(h + 1) * BT],
                            w2abf[:, 32 * p:32 * p + 32],
                            h1a[:, p * B + ch: p * B + ch + BT],
                            start=True, stop=False,
                            tile_position=(0, 32 * q),
                            skip_group_check=True)
            # B contributions: one block-diagonal matmul per (g, half)
            for g in range(2):
                for h in range(2):
                    ch = c0 + h * BT
                    nc.tensor.matmul(
                        ps2g[g][:, h * BT:(h + 1) * BT],
                        w2bbf[:, 128 * g:128 * g + 128],
                        h1b[:, g * B + ch: g * B + ch + BT],
                        start=False, stop=True,
                        skip_group_check=True)
            for g in range(2):
                h2s = h2[:, g * B + c0: g * B + c0 + 2 * BT]
                nc.scalar.activation(h2s, ps2g[g][:], TANH, bias=b2p[:, g:g + 1])
                for h in range(2):
                    bt = 2 * b2 + h
                    nc.vector.bn_stats(
                        st2[:, g * 48 + bt * 6: g * 48 + bt * 6 + 6],
                        h2[:, g * B + bt * BT: g * B + (bt + 1) * BT])
        sbH1.release()

        # ------------------------------------------------ L2 stats -> fold
        for g in range(2):
            nc.vector.bn_aggr(agg2[:, 2 * g:2 * g + 2], st2[:, g * 48:(g + 1) * 48])
        nc.vector.tensor_scalar(out=nsS[:, 10:12], in0=agg2[:, 1::2],
                                scalar1=EPS, scalar2=None, op0=AL.add)
        _rsqrt_newton(nc, AL, nsY[:, 10:12], nsS[:, 10:12],
                      nsT[:, 10:12], magic[:, 10:12])
        nc.vector.tensor_tensor(out=a2[:], in0=nsY[:, 10:12],
                                in1=s96sb[:, 64:66], op=AL.mult)
        nc.vector.tensor_tensor(out=ctm[:, 2:4], in0=agg2[:, 0::2], in1=a2[:],
                                op=AL.mult)
        nc.vector.tensor_tensor(out=c2[:], in0=s96sb[:, 66:68],
                                in1=ctm[:, 2:4], op=AL.subtract)
        nc.vector.tensor_scalar(out=w3abf[:], in0=s96sb[:, 0:32],
                                scalar1=a2[:, 0:1], scalar2=None, op0=AL.mult)
        nc.vector.tensor_scalar(out=w3bbf[:], in0=s96sb[:, 32:64],
                                scalar1=a2[:, 1:2], scalar2=None, op0=AL.mult)
        psT3 = psmm.tile([32, 1], f32, name="psT3", tag="mm")
        nc.tensor.matmul(psT3[:], s96sb[:, 0:32], c2[:, 0:1], start=True, stop=False)
        nc.tensor.matmul(psT3[:], s96sb[:, 32:64], c2[:, 1:2], start=False, stop=True)
        nc.scalar.activation(b3sb[:], psT3[:], IDENT, bias=s32sb[:, 64:65])

        # ------------------------------------------------ level 3
        sbH3 = tc.alloc_tile_pool(name="sbH3", bufs=1)
        h3 = sbH3.tile([32, B], bf16, name="h3")
        for b2 in range(NBT // 2):
            c0 = b2 * 2 * BT
            ps3 = psmm.tile([32, 2 * BT], f32, name=f"ps3_{b2}", tag="mm")
            for h in range(2):
                nc.tensor.matmul(ps3[:, h * BT:(h + 1) * BT], w3abf[:],
                                 h2[:, c0 + h * BT: c0 + (h + 1) * BT],
                                 start=True, stop=False)
            for h in range(2):
                nc.tensor.matmul(ps3[:, h * BT:(h + 1) * BT], w3bbf[:],
                                 h2[:, B + c0 + h * BT: B + c0 + (h + 1) * BT],
                                 start=False, stop=True)
            h3s = h3[:, c0:c0 + 2 * BT]
            nc.scalar.activation(h3s, ps3[:], TANH, bias=b3sb[:])
            for h in range(2):
                bt = 2 * b2 + h
                nc.vector.bn_stats(st3[:, bt * 6: bt * 6 + 6],
                                   h3[:, bt * BT:(bt + 1) * BT])
        sbH2.release()

        nc.vector.bn_aggr(agg3[:], st3[:])
        nc.vector.tensor_scalar(out=nsS[0:32, 12:13], in0=agg3[:, 1:2],
                                scalar1=EPS, scalar2=None, op0=AL.add)
        _rsqrt_newton(nc, AL, nsY[0:32, 12:13], nsS[0:32, 12:13],
                      nsT[0:32, 12:13], magic[0:32, 12:13])
        nc.vector.tensor_tensor(out=a3[:], in0=nsY[0:32, 12:13],
                                in1=s32sb[:, 65:66], op=AL.mult)
        nc.vector.tensor_tensor(out=ctm[0:32, 4:5], in0=agg3[:, 0:1], in1=a3[:],
                                op=AL.mult)
        nc.vector.tensor_tensor(out=c3[:], in0=s32sb[:, 66:67],
                                in1=ctm[0:32, 4:5], op=AL.subtract)
        nc.vector.tensor_scalar(out=wrbf[:], in0=s32sb[:, 0:64], scalar1=a3[:],
                                scalar2=None, op0=AL.mult)
        psT4 = psmm.tile([64, 1], f32, name="psT4", tag="mm")
        nc.tensor.matmul(psT4[:], s32sb[:, 0:64], c3[:], start=True, stop=True)
        nc.scalar.copy(brc[:], psT4[:])

        # ------------- root partial + bf16 AllReduce + pid-sliced tail
        sbT = tc.alloc_tile_pool(name="sbT", bufs=1, side="right")
        partial = sbT.tile([64, B], bf16, name="partial")
        BS = B // N_CORES
        for b2 in range(NBT // 2):
            c0 = b2 * 2 * BT
            psr = psmm.tile([64, 2 * BT], f32, name=f"psr_{b2}", tag="mm")
            for h in range(2):
                nc.tensor.matmul(psr[:, h * BT:(h + 1) * BT], wrbf[:],
                                 h3[:, c0 + h * BT: c0 + (h + 1) * BT],
                                 start=True, stop=True)
            nc.scalar.activation(partial[:, c0:c0 + 2 * BT], psr[:], IDENT,
                                 bias=brc[:])
            # ship each chunk to the collective buffer as it is produced
            nc.sync.dma_start(cc_in[:, c0:c0 + 2 * BT],
                              partial[:, c0:c0 + 2 * BT])
        red = sbT.tile([64, B], bf16, name="red")
        hr = sbT.tile([64, B], f32, name="hr")
        outTc = sbT.tile([64, BS], f32, name="outTc")
        outSc = sbT.tile([128, BS // 2], f32, name="outSc")
        nc.gpsimd.collective_compute(
            "AllReduce", AL.add,
            replica_groups=[list(range(N_CORES))],
            ins=[cc_in[:].opt()], outs=[cc_out[:].opt()])
        nc.sync.dma_start(red[:], cc_out[:])
        for k in range(2):
            c0 = k * (B // 2)
            nc.scalar.activation(hr[:, c0:c0 + B // 2], red[:, c0:c0 + B // 2],
                                 TANH, bias=s64sb[:, 0:1])
            for j in range(4):
                bt = 4 * k + j
                nc.vector.bn_stats(stR[:, bt * 6: bt * 6 + 6],
                                   hr[:, bt * BT:(bt + 1) * BT])
        nc.vector.bn_aggr(aggR[:], stR[:])
        nc.vector.tensor_scalar(out=nsS[0:64, 13:14], in0=aggR[:, 1:2],
                                scalar1=EPS, scalar2=None, op0=AL.add)
        _rsqrt_newton(nc, AL, nsY[0:64, 13:14], nsS[0:64, 13:14],
                      nsT[0:64, 13:14], magic[0:64, 13:14])
        nc.vector.tensor_tensor(out=srt[:], in0=nsY[0:64, 13:14],
                                in1=s64sb[:, 1:2], op=AL.mult)
        nc.vector.tensor_tensor(out=ctm[0:64, 5:6], in0=aggR[:, 0:1], in1=srt[:],
                                op=AL.mult)
        nc.vector.tensor_tensor(out=drt[:], in0=s64sb[:, 2:3],
                                in1=ctm[0:64, 5:6], op=AL.subtract)
        # each core normalizes + writes only its own 512-row batch slice
        import concourse.bass as bass_mod
        pid = nc.vector.partition_id()
        off = pid * BS
        nc.vector.tensor_scalar(out=outTc[:],
                                in0=hr[:, bass_mod.ds(off, BS)],
                                scalar1=srt[:], scalar2=drt[:],
                                op0=AL.mult, op1=AL.add)
        # transpose [64, 512] -> [512, 64] via PE, 128 batch rows at a time
        for t in range(BS // 128):
            pstr = psmm.tile([128, 64], f32, name=f"pstr_{t}", tag="mm")
            nc.tensor.transpose(pstr[:], outTc[:, t * 128:(t + 1) * 128],
                                s64sb[:, 3:67])
            nc.vector.tensor_copy(outSc[:, t * 64:(t + 1) * 64], pstr[:])
        nc.sync.dma_start(outd[:].rearrange("(t p) o -> p t o", p=128),
                          outSc[:].rearrange("p (t o) -> p t o", o=64))

        sbT.release()
        sbH3.release()
        sbS.release()
        psmm.release()

    nc.compile()
    return nc


# ---------------------------------------------------------------- host side

def shard_inputs(mutant_state, gene_idx, W1, b1, g1, beta1, W2, b2, g2, beta2,
                 W3, b3, g3, beta3, Wr, br, gr, betar):
    """Build the per-core in_maps."""
    mutant_state = np.asarray(mutant_state, dtype=np.float32)
    gene_idx = np.asarray(gene_idx)
    W1 = np.asarray(W1, np.float32); b1 = np.asarray(b1, np.float32)
    g1 = np.asarray(g1, np.float32); beta1 = np.asarray(beta1, np.float32)
    W2 = np.asarray(W2, np.float32); b2 = np.asarray(b2, np.float32)
    g2 = np.asarray(g2, np.float32); beta2 = np.asarray(beta2, np.float32)
    W3 = np.asarray(W3, np.float32); b3 = np.asarray(b3, np.float32)
    g3 = np.asarray(g3, np.float32); beta3 = np.asarray(beta3, np.float32)
    Wr = np.asarray(Wr, np.float32); br = np.asarray(br, np.float32)
    gr = np.asarray(gr, np.float32); betar = np.asarray(betar, np.float32)

    MT = np.ascontiguousarray(mutant_state.astype(BF16).T)  # [N, B] bf16
    eye = np.eye(64, dtype=np.float32)

    in_maps = []
    for c in range(N_CORES):
        idx = gene_idx[64 * c:64 * (c + 1)].reshape(8, 128)
        xg = np.ascontiguousarray(MT[idx])                 # [8, 128, B] bf16

        W1c = W1[64 * c:64 * (c + 1)].reshape(8, 8, 20, 16)
        blk = np.zeros((8, 128, 160), np.float32)
        for sl in range(8):
            blk[:, 16 * sl:16 * (sl + 1), 20 * sl:20 * (sl + 1)] = \
                W1c[:, sl].transpose(0, 2, 1)
        w1 = np.ascontiguousarray(
            blk.transpose(1, 0, 2).reshape(128, 1280)).astype(BF16)

        def sAcols(v):  # per-subsystem vec [64, 20] -> A [128,8], Bpack [128,2]
            vb = v[64 * c:64 * (c + 1)].reshape(8, 160)
            A = np.ascontiguousarray(vb[:, :128].T)
            Bp = np.ascontiguousarray(
                vb[:, 128:].reshape(2, 4, 32).transpose(1, 2, 0).reshape(128, 2))
            return A, Bp

        b1A, b1B = sAcols(b1); g1A, g1B = sAcols(g1); be1A, be1B = sAcols(beta1)
        sa = np.concatenate([b1A, g1A, be1A, b1B, g1B, be1B], axis=1)  # [128,30]

        # W2: lhsT layouts, 24 out-features zero-padded to 32-row slots
        W2c = W2[8 * c:8 * (c + 1)]                                    # [8,24,160]
        w2a = np.zeros((128, 8, 32), np.float32)
        w2a[:, :, :24] = W2c[:, :, :128].transpose(2, 0, 1)            # [128,8,24]
        w2a = w2a.reshape(128, 256)
        # block-diagonal per group: rows 32q+r = parent 4g+q's tail input
        # feature 128+r; cols 32q+o = that parent's (padded) output feature o
        w2b = np.zeros((128, 2, 128), np.float32)
        for g in range(2):
            for q in range(4):
                w2b[32 * q:32 * q + 32, g, 32 * q:32 * q + 24] = \
                    W2c[4 * g + q, :, 128:].T
        w2b = np.ascontiguousarray(
            np.concatenate([w2b[:, 0, :], w2b[:, 1, :]], axis=1))

        def pack128(v):  # [8, 24] per-parent -> [128, 2] padded 32-slots
            out = np.zeros((2, 4, 32), np.float32)
            out[:, :, :24] = v[8 * c:8 * (c + 1)].reshape(2, 4, 24)
            return np.ascontiguousarray(out.transpose(1, 2, 0).reshape(128, 2))

        b2h = pack128(b2)
        # W3: input features padded 24->32 per L2 parent: [192,32] -> [256,32]
        W3T = W3[c].T                                                  # [192, 32]
        W3pad = np.zeros((8, 32, 32), np.float32)
        W3pad[:, :24, :] = W3T.reshape(8, 24, 32)
        W3pad = W3pad.reshape(256, 32)
        s96 = np.concatenate([W3pad[:128], W3pad[128:], pack128(g2),
                              pack128(beta2)], axis=1)                 # [128, 68]
        s32 = np.concatenate([np.ascontiguousarray(Wr[:, 32 * c:32 * (c + 1)].T),
                              b3[c][:, None], g3[c][:, None], beta3[c][:, None]],
                             axis=1)                                   # [32, 67]
        s64 = np.concatenate([br[:, None], gr[:, None], betar[:, None], eye],
                             axis=1)                                   # [64, 67]

        in_maps.append({
            "xg": xg,
            "w1": w1,
            "w2a": np.ascontiguousarray(w2a),
            "w2b": np.ascontiguousarray(w2b),
            "sa": np.ascontiguousarray(sa),
            "s96": np.ascontiguousarray(s96),
            "s32": np.ascontiguousarray(s32),
            "s64": np.ascontiguousarray(s64),
            "b2h": b2h,
        })
    return in_maps


def get_program():
    global _PROG
    if _PROG is None:
        _PROG = build_program()
    return _PROG


def kernel(trace=False, **inputs):
    from concourse.bass_utils import run_bass_kernel_spmd
    nc = get_program()
    in_maps = shard_inputs(**inputs)
    res = run_bass_kernel_spmd(nc, in_maps, core_ids=list(range(N_CORES)),
                               trace=trace)
    out = np.concatenate([np.asarray(res.results[c]["out"], dtype=np.float32)
                          for c in range(N_CORES)], axis=0)
    if trace:
        kernel.last_result = res
    return out



# revision 2
# speedup vs baseline: 1.0996x; 1.0996x over previous
"""DCell-style hierarchical NN (gather -> 3x [Linear+Tanh+BatchNorm] -> root)
on 8 Trainium2 NeuronCores.

Sharding: tree-sharding ("model parallel over subsystems") — core c owns
L1 subsystems [64c, 64c+64), L2 parents [8c, 8c+8), L3 parent c, and the
FULL batch B=4096.  All BatchNorm statistics at levels 1-3 are then
core-local (full batch present on every core); the only cross-core
communication is a single 1 MB AllReduce of the root layer's partial
products.  BatchNorm is folded at runtime into the next level's weights:
with h = tanh(W x + b) kept pre-BN, o = a*h + c where
a = g*rsqrt(var+eps), c = beta - mean*a, so the next level uses
W' = W*a (per input column) and b' = b + W@c.

Device layout: features on partitions, batch on the free axis.  Matmul
operands are bf16 (PE runs 1 cycle/row vs 4 for fp32), accumulation /
statistics / normalization all fp32.  L1 uses block-diagonal weights
(8 subsystems of 16->20 per K=128 matmul).  L2 outputs are packed four
parents per PSUM bank in 32-row slots (24 real features zero-padded to
32 — PE column tiles must be 32-aligned).  Stats use bn_stats/bn_aggr
(one DVE pass for mean+var); rsqrt is a Newton iteration on the DVE to
avoid switching the ACT table set away from Tanh.  Matmuls of the same
PE tiling mode are batched to avoid mode-switch drains.
"""

import numpy as np
import ml_dtypes

BF16 = ml_dtypes.bfloat16
N_CORES = 8
B = 4096
BT = 512
NBT = B // BT
EPS = 1e-5
MAGIC = 0x5F3759DF

_PROG = None  # cached program across calls


# ---------------------------------------------------------------- device code

def _rsqrt_newton(nc, AL, y, s, t, magic, iters=3):
    """y = rsqrt(s), all APs same shape, f32 (magic: int32)."""
    import concourse.mybir as mybir
    i32 = mybir.dt.int32
    nc.vector.tensor_scalar(out=t.bitcast(i32), in0=s.bitcast(i32),
                            scalar1=1, scalar2=None, op0=AL.arith_shift_right)
    nc.vector.tensor_tensor(out=y.bitcast(i32), in0=magic, in1=t.bitcast(i32),
                            op=AL.subtract)
    for _ in range(iters):
        nc.vector.tensor_tensor(out=t, in0=y, in1=y, op=AL.mult)
        nc.vector.tensor_tensor(out=t, in0=t, in1=s, op=AL.mult)
        nc.vector.tensor_scalar(out=t, in0=t, scalar1=-0.5, scalar2=1.5,
                                op0=AL.mult, op1=AL.add)
        nc.vector.tensor_tensor(out=y, in0=y, in1=t, op=AL.mult)


def build_program():
    import concourse.bacc as bacc
    import concourse.mybir as mybir
    import concourse.tile as tile

    f32 = mybir.dt.float32
    bf16 = mybir.dt.bfloat16
    i32 = mybir.dt.int32
    AL = mybir.AluOpType
    TANH = mybir.ActivationFunctionType.Tanh
    IDENT = mybir.ActivationFunctionType.Identity

    nc = bacc.Bacc("TRN2", target_bir_lowering=False, debug=False,
                   num_devices=N_CORES)

    # ------------------------------------------------ DRAM I/O (per core)
    xgd = nc.dram_tensor("xg", [8, 128, B], bf16, kind="ExternalInput")
    w1d = nc.dram_tensor("w1", [128, 1280], bf16, kind="ExternalInput")
    w2ad = nc.dram_tensor("w2a", [128, 256], f32, kind="ExternalInput")
    w2bd = nc.dram_tensor("w2b", [128, 256], f32, kind="ExternalInput")
    sad = nc.dram_tensor("sa", [128, 30], f32, kind="ExternalInput")
    s96d = nc.dram_tensor("s96", [128, 68], f32, kind="ExternalInput")
    s32d = nc.dram_tensor("s32", [32, 67], f32, kind="ExternalInput")
    s64d = nc.dram_tensor("s64", [64, 67], f32, kind="ExternalInput")
    b2hd = nc.dram_tensor("b2h", [128, 2], f32, kind="ExternalInput")
    outd = nc.dram_tensor("out", [B // N_CORES, 64], f32, kind="ExternalOutput")
    cc_in = nc.dram_tensor("cc_in", [64, B], bf16)
    cc_out = nc.dram_tensor("cc_out", [64, B], bf16, addr_space="Shared")

    with tile.TileContext(nc) as tc:
        sbS = tc.alloc_tile_pool(name="sbS", bufs=1)
        sbH1 = tc.alloc_tile_pool(name="sbH1", bufs=1)
        sbX = tc.alloc_tile_pool(name="sbX", bufs=1, side="right")
        psmm = tc.alloc_tile_pool(name="psmm", bufs=4, space="PSUM")

        # static tiles
        w1sb = sbS.tile([128, 1280], bf16, name="w1sb")
        w2asb = sbS.tile([128, 256], f32, name="w2asb")
        w2bsb = sbS.tile([128, 256], f32, name="w2bsb")
        sAsb = sbS.tile([128, 30], f32, name="sAsb")
        s96sb = sbS.tile([128, 68], f32, name="s96sb")
        s32sb = sbS.tile([32, 67], f32, name="s32sb")
        s64sb = sbS.tile([64, 67], f32, name="s64sb")
        b2hsb = sbS.tile([128, 2], f32, name="b2hsb")
        stA = sbS.tile([128, 384], f32, name="stA")   # 8 parents * 8bt * 6
        stB = sbS.tile([128, 96], f32, name="stB")    # 2 groups
        st2 = sbS.tile([128, 96], f32, name="st2")
        st3 = sbS.tile([32, 48], f32, name="st3")
        stR = sbS.tile([64, 48], f32, name="stR")
        aggA = sbS.tile([128, 16], f32, name="aggA")
        aggB = sbS.tile([128, 4], f32, name="aggB")
        agg2 = sbS.tile([128, 4], f32, name="agg2")
        agg3 = sbS.tile([32, 2], f32, name="agg3")
        aggR = sbS.tile([64, 2], f32, name="aggR")
        magic = sbS.tile([128, 16], i32, name="magic")
        nsS = sbS.tile([128, 16], f32, name="nsS")
        nsT = sbS.tile([128, 16], f32, name="nsT")
        nsY = sbS.tile([128, 16], f32, name="nsY")
        ctm = sbS.tile([128, 8], f32, name="ctm")
        aA = sbS.tile([128, 8], f32, name="aA")
        cA = sbS.tile([128, 8], f32, name="cA")
        aB = sbS.tile([128, 2], f32, name="aB")
        cB = sbS.tile([128, 2], f32, name="cB")
        a2 = sbS.tile([128, 2], f32, name="a2")
        c2 = sbS.tile([128, 2], f32, name="c2")
        a3 = sbS.tile([32, 1], f32, name="a3")
        c3 = sbS.tile([32, 1], f32, name="c3")
        srt = sbS.tile([64, 1], f32, name="srt")
        drt = sbS.tile([64, 1], f32, name="drt")
        w2abf = sbS.tile([128, 256], bf16, name="w2abf")
        w2bbf = sbS.tile([128, 256], bf16, name="w2bbf")
        w3abf = sbS.tile([128, 32], bf16, name="w3abf")
        w3bbf = sbS.tile([128, 32], bf16, name="w3bbf")
        wrbf = sbS.tile([32, 64], bf16, name="wrbf")
        b2p = sbS.tile([128, 2], f32, name="b2p")
        b3sb = sbS.tile([32, 1], f32, name="b3sb")
        brc = sbS.tile([64, 1], f32, name="brc")

        h1a = sbH1.tile([128, 8 * B], bf16, name="h1a")
        h1b = sbH1.tile([128, 2 * B], bf16, name="h1b")
        xsb = sbX.tile([128, 8 * B], bf16, name="xsb")

        nc.vector.memset(magic[:], MAGIC)

        # ------------------------------------------------ input DMAs
        # small params first (they gate the first activations), then the
        # bulk gathered input chunk by chunk so L1 compute overlaps the
        # remaining transfers.
        nc.sync.dma_start(sAsb[:], sad[:])
        nc.sync.dma_start(w1sb[:], w1d[:])
        nc.sync.dma_start(b2hsb[:], b2hd[:])
        nc.sync.dma_start(w2asb[:], w2ad[:])
        nc.sync.dma_start(w2bsb[:], w2bd[:])
        nc.sync.dma_start(s96sb[:], s96d[:])
        nc.sync.dma_start(s32sb[:], s32d[:])
        nc.sync.dma_start(s64sb[:], s64d[:])
        for p in range(8):
            nc.sync.dma_start(xsb[:, p * B:(p + 1) * B], xgd[p, :, :])

        # ------------------------------------------------ level 1
        # A pass: per-parent [K=128 genes] -> [128 features] (mode 128x128),
        # weight-stationary (p outer), batch-tile pairs share a 2-bank psum
        # so each ACT covers [128, 1024].  B pass (remaining 32 features x 4
        # parents packed per bank, mode 128x32) is interleaved per group so
        # its stats finish early.
        def l1_a(p):
            for b2 in range(NBT // 2):
                c0 = b2 * 2 * BT
                psA = psmm.tile([128, 2 * BT], f32, name=f"psA_{p}_{b2}",
                                tag="mm")
                for h in range(2):
                    nc.tensor.matmul(
                        psA[:, h * BT:(h + 1) * BT],
                        w1sb[:, p * 160: p * 160 + 128],
                        xsb[:, p * B + c0 + h * BT: p * B + c0 + (h + 1) * BT],
                        start=True, stop=True)
                ha = h1a[:, p * B + c0: p * B + c0 + 2 * BT]
                nc.scalar.activation(ha, psA[:], TANH, bias=sAsb[:, p:p + 1])
                for h in range(2):
                    bt = 2 * b2 + h
                    nc.vector.bn_stats(
                        stA[:, p * 48 + bt * 6: p * 48 + bt * 6 + 6],
                        h1a[:, p * B + bt * BT: p * B + (bt + 1) * BT])
            nc.vector.bn_aggr(aggA[:, 2 * p:2 * p + 2],
                              stA[:, p * 48:(p + 1) * 48])

        def l1_b(g, b2s):
            for b2 in b2s:
                c0 = b2 * 2 * BT
                psB = psmm.tile([128, 2 * BT], f32, name=f"psB_{b2}_{g}",
                                tag="mm")
                for h in range(2):
                    ch = c0 + h * BT
                    for q in range(4):
                        p = 4 * g + q
                        nc.tensor.matmul(
                            psB[32 * q:32 * q + 32, h * BT:(h + 1) * BT],
                            w1sb[:, p * 160 + 128:(p + 1) * 160],
                            xsb[:, p * B + ch: p * B + ch + BT],
                            start=True, stop=True,
                            tile_position=(0, 32 * q),
                            skip_group_check=True)
                hb = h1b[:, g * B + c0: g * B + c0 + 2 * BT]
                nc.scalar.activation(hb, psB[:], TANH,
                                     bias=sAsb[:, 24 + g:25 + g])
                for h in range(2):
                    bt = 2 * b2 + h
                    nc.vector.bn_stats(
                        stB[:, g * 48 + bt * 6: g * 48 + bt * 6 + 6],
                        h1b[:, g * B + bt * BT: g * B + (bt + 1) * BT])

        for p in range(4):
            l1_a(p)
        l1_b(0, range(0, 2))
        l1_a(4)
        l1_b(0, range(2, 4))
        nc.vector.bn_aggr(aggB[:, 0:2], stB[:, 0:48])
        for p in range(5, 8):
            l1_a(p)
        l1_b(1, range(0, 4))
        nc.vector.bn_aggr(aggB[:, 2:4], stB[:, 48:96])
        sbX.release()

        # ------------------------------------------------ L1 stats -> fold
        nc.vector.tensor_scalar(out=nsS[:, 0:8], in0=aggA[:, 1::2],
                                scalar1=EPS, scalar2=None, op0=AL.add)
        _rsqrt_newton(nc, AL, nsY[:, 0:8], nsS[:, 0:8], nsT[:, 0:8], magic[:, 0:8])
        nc.vector.tensor_tensor(out=aA[:], in0=nsY[:, 0:8], in1=sAsb[:, 8:16],
                                op=AL.mult)
        nc.vector.tensor_tensor(out=ctm[:], in0=aggA[:, 0::2], in1=aA[:], op=AL.mult)
        nc.vector.tensor_tensor(out=cA[:], in0=sAsb[:, 16:24], in1=ctm[:],
                                op=AL.subtract)
        nc.vector.tensor_scalar(out=nsS[:, 8:10], in0=aggB[:, 1::2],
                                scalar1=EPS, scalar2=None, op0=AL.add)
        _rsqrt_newton(nc, AL, nsY[:, 8:10], nsS[:, 8:10], nsT[:, 8:10], magic[:, 8:10])
        nc.vector.tensor_tensor(out=aB[:], in0=nsY[:, 8:10], in1=sAsb[:, 26:28],
                                op=AL.mult)
        nc.vector.tensor_tensor(out=ctm[:, 0:2], in0=aggB[:, 0::2], in1=aB[:],
                                op=AL.mult)
        nc.vector.tensor_tensor(out=cB[:], in0=sAsb[:, 28:30], in1=ctm[:, 0:2],
                                op=AL.subtract)
        # fold BN1 into W2 (bf16) and bias.  w2b is block-diagonal per group
        # ([128, 128] covering 4 parents), so its fold is one op per group.
        for p in range(8):
            nc.vector.tensor_scalar(out=w2abf[:, 32 * p:32 * p + 32],
                                    in0=w2asb[:, 32 * p:32 * p + 32],
                                    scalar1=aA[:, p:p + 1], scalar2=None,
                                    op0=AL.mult)
        for g in range(2):
            nc.vector.tensor_scalar(out=w2bbf[:, 128 * g:128 * g + 128],
                                    in0=w2bsb[:, 128 * g:128 * g + 128],
                                    scalar1=aB[:, g:g + 1], scalar2=None,
                                    op0=AL.mult)
        psT2 = [psmm.tile([128, BT], f32, name=f"psb2_{g}", tag="mm")
                for g in range(2)]
        for g in range(2):
            for q in range(4):
                p = 4 * g + q
                nc.tensor.matmul(psT2[g][32 * q:32 * q + 32, 0:1],
                                 w2asb[:, 32 * p:32 * p + 32], cA[:, p:p + 1],
                                 start=True, stop=False,
                                 tile_position=(0, 32 * q),
                                 skip_group_check=True)
        for g in range(2):
            nc.tensor.matmul(psT2[g][:, 0:1], w2bsb[:, 128 * g:128 * g + 128],
                             cB[:, g:g + 1], start=False, stop=True,
                             skip_group_check=True)
        for g in range(2):
            nc.scalar.activation(b2p[:, g:g + 1], psT2[g][:, 0:1], IDENT,
                                 bias=b2hsb[:, g:g + 1])

        # ------------------------------------------------ level 2
        sbH2 = tc.alloc_tile_pool(name="sbH2", bufs=1, side="right")
        h2 = sbH2.tile([128, 2 * B], bf16, name="h2")
        for b2 in range(NBT // 2):
            c0 = b2 * 2 * BT
            ps2g = [psmm.tile([128, 2 * BT], f32, name=f"ps2_{b2}_{g}",
                              tag="mm") for g in range(2)]
            # A contributions: (128x32) col-tiled mode; both batch halves
            # back-to-back so each stationary weight is loaded once
            for g in range(2):
                for q in range(4):
                    p = 4 * g + q
                    for h in range(2):
                        ch = c0 + h * BT
                        nc.tensor.matmul(
                            ps2g[g][32 * q:32 * q + 32, h * BT:(h + 1) * BT],
                            w2abf[:, 32 * p:32 * p + 32],
                            h1a[:, p * B + ch: p * B + ch + BT],
                            start=True, stop=False,
                            tile_position=(0, 32 * q),
                            skip_group_check=True)
            # B contributions: one block-diagonal matmul per (g, half)
            for g in range(2):
                for h in range(2):
                    ch = c0 + h * BT
                    nc.tensor.matmul(
                        ps2g[g][:, h * BT:(h + 1) * BT],
                        w2bbf[:, 128 * g:128 * g + 128],
                        h1b[:, g * B + ch: g * B + ch + BT],
                        start=False, stop=True,
                        skip_group_check=True)
            for g in range(2):
                h2s = h2[:, g * B + c0: g * B + c0 + 2 * BT]
                nc.scalar.activation(h2s, ps2g[g][:], TANH, bias=b2p[:, g:g + 1])
                for h in range(2):
                    bt = 2 * b2 + h
                    nc.vector.bn_stats(
                        st2[:, g * 48 + bt * 6: g * 48 + bt * 6 + 6],
                        h2[:, g * B + bt * BT: g * B + (bt + 1) * BT])
        sbH1.release()

        # ------------------------------------------------ L2 stats -> fold
        for g in range(2):
            nc.vector.bn_aggr(agg2[:, 2 * g:2 * g + 2], st2[:, g * 48:(g + 1) * 48])
        nc.vector.tensor_scalar(out=nsS[:, 10:12], in0=agg2[:, 1::2],
                                scalar1=EPS, scalar2=None, op0=AL.add)
        _rsqrt_newton(nc, AL, nsY[:, 10:12], nsS[:, 10:12],
                      nsT[:, 10:12], magic[:, 10:12])
        nc.vector.tensor_tensor(out=a2[:], in0=nsY[:, 10:12],
                                in1=s96sb[:, 64:66], op=AL.mult)
        nc.vector.tensor_tensor(out=ctm[:, 2:4], in0=agg2[:, 0::2], in1=a2[:],
                                op=AL.mult)
        nc.vector.tensor_tensor(out=c2[:], in0=s96sb[:, 66:68],
                                in1=ctm[:, 2:4], op=AL.subtract)
        nc.vector.tensor_scalar(out=w3abf[:], in0=s96sb[:, 0:32],
                                scalar1=a2[:, 0:1], scalar2=None, op0=AL.mult)
        nc.vector.tensor_scalar(out=w3bbf[:], in0=s96sb[:, 32:64],
                                scalar1=a2[:, 1:2], scalar2=None, op0=AL.mult)
        psT3 = psmm.tile([32, 1], f32, name="psT3", tag="mm")
        nc.tensor.matmul(psT3[:], s96sb[:, 0:32], c2[:, 0:1], start=True, stop=False)
        nc.tensor.matmul(psT3[:], s96sb[:, 32:64], c2[:, 1:2], start=False, stop=True)
        nc.scalar.activation(b3sb[:], psT3[:], IDENT, bias=s32sb[:, 64:65])

        # ------------------------------------------------ level 3
        sbH3 = tc.alloc_tile_pool(name="sbH3", bufs=1)
        h3 = sbH3.tile([32, B], bf16, name="h3")
        for b2 in range(NBT // 2):
            c0 = b2 * 2 * BT
            ps3 = psmm.tile([32, 2 * BT], f32, name=f"ps3_{b2}", tag="mm")
            for h in range(2):
                nc.tensor.matmul(ps3[:, h * BT:(h + 1) * BT], w3abf[:],
                                 h2[:, c0 + h * BT: c0 + (h + 1) * BT],
                                 start=True, stop=False)
            for h in range(2):
                nc.tensor.matmul(ps3[:, h * BT:(h + 1) * BT], w3bbf[:],
                                 h2[:, B + c0 + h * BT: B + c0 + (h + 1) * BT],
                                 start=False, stop=True)
            h3s = h3[:, c0:c0 + 2 * BT]
            nc.scalar.activation(h3s, ps3[:], TANH, bias=b3sb[:])
            for h in range(2):
                bt = 2 * b2 + h
                nc.vector.bn_stats(st3[:, bt * 6: bt * 6 + 6],
                                   h3[:, bt * BT:(bt + 1) * BT])
        sbH2.release()

        nc.vector.bn_aggr(agg3[:], st3[:])
        nc.vector.tensor_scalar(out=nsS[0:32, 12:13], in0=agg3[:, 1:2],
                                scalar1=EPS, scalar2=None, op0=AL.add)
        _rsqrt_newton(nc, AL, nsY[0:32, 12:13], nsS[0:32, 12:13],
                      nsT[0:32, 12:13], magic[0:32, 12:13])
        nc.vector.tensor_tensor(out=a3[:], in0=nsY[0:32, 12:13],
                                in1=s32sb[:, 65:66], op=AL.mult)
        nc.vector.tensor_tensor(out=ctm[0:32, 4:5], in0=agg3[:, 0:1], in1=a3[:],
                                op=AL.mult)
        nc.vector.tensor_tensor(out=c3[:], in0=s32sb[:, 66:67],
                                in1=ctm[0:32, 4:5], op=AL.subtract)
        nc.vector.tensor_scalar(out=wrbf[:], in0=s32sb[:, 0:64], scalar1=a3[:],
                                scalar2=None, op0=AL.mult)
        psT4 = psmm.tile([64, 1], f32, name="psT4", tag="mm")
        nc.tensor.matmul(psT4[:], s32sb[:, 0:64], c3[:], start=True, stop=True)
        nc.scalar.copy(brc[:], psT4[:])

        # ------------- root partial + bf16 AllReduce + pid-sliced tail
        sbT = tc.alloc_tile_pool(name="sbT", bufs=1, side="right")
        partial = sbT.tile([64, B], bf16, name="partial")
        BS = B // N_CORES
        for b2 in range(NBT // 2):
            c0 = b2 * 2 * BT
            psr = psmm.tile([64, 2 * BT], f32, name=f"psr_{b2}", tag="mm")
            for h in range(2):
                nc.tensor.matmul(psr[:, h * BT:(h + 1) * BT], wrbf[:],
                                 h3[:, c0 + h * BT: c0 + (h + 1) * BT],
                                 start=True, stop=True)
            nc.scalar.activation(partial[:, c0:c0 + 2 * BT], psr[:], IDENT,
                                 bias=brc[:])
            # ship each chunk to the collective buffer as it is produced
            nc.sync.dma_start(cc_in[:, c0:c0 + 2 * BT],
                              partial[:, c0:c0 + 2 * BT])
        red = sbT.tile([64, B], bf16, name="red")
        hr = sbT.tile([64, B], f32, name="hr")
        outTc = sbT.tile([64, BS], f32, name="outTc")
        outSc = sbT.tile([128, BS // 2], f32, name="outSc")
        nc.gpsimd.collective_compute(
            "AllReduce", AL.add,
            replica_groups=[list(range(N_CORES))],
            ins=[cc_in[:].opt()], outs=[cc_out[:].opt()])
        nc.sync.dma_start(red[:], cc_out[:])
        for k in range(2):
            c0 = k * (B // 2)
            nc.scalar.activation(hr[:, c0:c0 + B // 2], red[:, c0:c0 + B // 2],
                                 TANH, bias=s64sb[:, 0:1])
            for j in range(4):
                bt = 4 * k + j
                nc.vector.bn_stats(stR[:, bt * 6: bt * 6 + 6],
                                   hr[:, bt * BT:(bt + 1) * BT])
        nc.vector.bn_aggr(aggR[:], stR[:])
        nc.vector.tensor_scalar(out=nsS[0:64, 13:14], in0=aggR[:, 1:2],
                                scalar1=EPS, scalar2=None, op0=AL.add)
        _rsqrt_newton(nc, AL, nsY[0:64, 13:14], nsS[0:64, 13:14],
                      nsT[0:64, 13:14], magic[0:64, 13:14])
        nc.vector.tensor_tensor(out=srt[:], in0=nsY[0:64, 13:14],
                                in1=s64sb[:, 1:2], op=AL.mult)
        nc.vector.tensor_tensor(out=ctm[0:64, 5:6], in0=aggR[:, 0:1], in1=srt[:],
                                op=AL.mult)
        nc.vector.tensor_tensor(out=drt[:], in0=s64sb[:, 2:3],
                                in1=ctm[0:64, 5:6], op=AL.subtract)
        # each core normalizes + writes only its own 512-row batch slice
        import concourse.bass as bass_mod
        pid = nc.vector.partition_id()
        off = pid * BS
        nc.vector.tensor_scalar(out=outTc[:],
                                in0=hr[:, bass_mod.ds(off, BS)],
                                scalar1=srt[:], scalar2=drt[:],
                                op0=AL.mult, op1=AL.add)
        # transpose [64, 512] -> [512, 64] via PE, 128 batch rows at a time
        for t in range(BS // 128):
            pstr = psmm.tile([128, 64], f32, name=f"pstr_{t}", tag="mm")
            nc.tensor.transpose(pstr[:], outTc[:, t * 128:(t + 1) * 128],
                                s64sb[:, 3:67])
            nc.vector.tensor_copy(outSc[:, t * 64:(t + 1) * 64], pstr[:])
        nc.sync.dma_start(outd[:].rearrange("(t p) o -> p t o", p=128),
                          outSc[:].rearrange("p (t o) -> p t o", o=64))

        sbT.release()
        sbH3.release()
        sbS.release()
        psmm.release()

    nc.compile()
    return nc


# ---------------------------------------------------------------- host side

def shard_inputs(mutant_state, gene_idx, W1, b1, g1, beta1, W2, b2, g2, beta2,
                 W3, b3, g3, beta3, Wr, br, gr, betar):
    """Build the per-core in_maps."""
    mutant_state = np.asarray(mutant_state, dtype=np.float32)
    gene_idx = np.asarray(gene_idx)
    W1 = np.asarray(W1, np.float32); b1 = np.asarray(b1, np.float32)
    g1 = np.asarray(g1, np.float32); beta1 = np.asarray(beta1, np.float32)
    W2 = np.asarray(W2, np.float32); b2 = np.asarray(b2, np.float32)
    g2 = np.asarray(g2, np.float32); beta2 = np.asarray(beta2, np.float32)
    W3 = np.asarray(W3, np.float32); b3 = np.asarray(b3, np.float32)
    g3 = np.asarray(g3, np.float32); beta3 = np.asarray(beta3, np.float32)
    Wr = np.asarray(Wr, np.float32); br = np.asarray(br, np.float32)
    gr = np.asarray(gr, np.float32); betar = np.asarray(betar, np.float32)

    MT = np.ascontiguousarray(mutant_state.astype(BF16).T)  # [N, B] bf16
    eye = np.eye(64, dtype=np.float32)

    in_maps = []
    for c in range(N_CORES):
        idx = gene_idx[64 * c:64 * (c + 1)].reshape(8, 128)
        xg = np.ascontiguousarray(MT[idx])                 # [8, 128, B] bf16

        W1c = W1[64 * c:64 * (c + 1)].reshape(8, 8, 20, 16)
        blk = np.zeros((8, 128, 160), np.float32)
        for sl in range(8):
            blk[:, 16 * sl:16 * (sl + 1), 20 * sl:20 * (sl + 1)] = \
                W1c[:, sl].transpose(0, 2, 1)
        w1 = np.ascontiguousarray(
            blk.transpose(1, 0, 2).reshape(128, 1280)).astype(BF16)

        def sAcols(v):  # per-subsystem vec [64, 20] -> A [128,8], Bpack [128,2]
            vb = v[64 * c:64 * (c + 1)].reshape(8, 160)
            A = np.ascontiguousarray(vb[:, :128].T)
            Bp = np.ascontiguousarray(
                vb[:, 128:].reshape(2, 4, 32).transpose(1, 2, 0).reshape(128, 2))
            return A, Bp

        b1A, b1B = sAcols(b1); g1A, g1B = sAcols(g1); be1A, be1B = sAcols(beta1)
        sa = np.concatenate([b1A, g1A, be1A, b1B, g1B, be1B], axis=1)  # [128,30]

        # W2: lhsT layouts, 24 out-features zero-padded to 32-row slots
        W2c = W2[8 * c:8 * (c + 1)]                                    # [8,24,160]
        w2a = np.zeros((128, 8, 32), np.float32)
        w2a[:, :, :24] = W2c[:, :, :128].transpose(2, 0, 1)            # [128,8,24]
        w2a = w2a.reshape(128, 256)
        # block-diagonal per group: rows 32q+r = parent 4g+q's tail input
        # feature 128+r; cols 32q+o = that parent's (padded) output feature o
        w2b = np.zeros((128, 2, 128), np.float32)
        for g in range(2):
            for q in range(4):
                w2b[32 * q:32 * q + 32, g, 32 * q:32 * q + 24] = \
                    W2c[4 * g + q, :, 128:].T
        w2b = np.ascontiguousarray(
            np.concatenate([w2b[:, 0, :], w2b[:, 1, :]], axis=1))

        def pack128(v):  # [8, 24] per-parent -> [128, 2] padded 32-slots
            out = np.zeros((2, 4, 32), np.float32)
            out[:, :, :24] = v[8 * c:8 * (c + 1)].reshape(2, 4, 24)
            return np.ascontiguousarray(out.transpose(1, 2, 0).reshape(128, 2))

        b2h = pack128(b2)
        # W3: input features padded 24->32 per L2 parent: [192,32] -> [256,32]
        W3T = W3[c].T                                                  # [192, 32]
        W3pad = np.zeros((8, 32, 32), np.float32)
        W3pad[:, :24, :] = W3T.reshape(8, 24, 32)
        W3pad = W3pad.reshape(256, 32)
        s96 = np.concatenate([W3pad[:128], W3pad[128:], pack128(g2),
                              pack128(beta2)], axis=1)                 # [128, 68]
        s32 = np.concatenate([np.ascontiguousarray(Wr[:, 32 * c:32 * (c + 1)].T),
                              b3[c][:, None], g3[c][:, None], beta3[c][:, None]],
                             axis=1)                                   # [32, 67]
        s64 = np.concatenate([br[:, None], gr[:, None], betar[:, None], eye],
                             axis=1)                                   # [64, 67]

        in_maps.append({
            "xg": xg,
            "w1": w1,
            "w2a": np.ascontiguousarray(w2a),
            "w2b": np.ascontiguousarray(w2b),
            "sa": np.ascontiguousarray(sa),
            "s96": np.ascontiguousarray(s96),
            "s32": np.ascontiguousarray(s32),
            "s64": np.ascontiguousarray(s64),
            "b2h": b2h,
        })
    return in_maps


def get_program():
    global _PROG
    if _PROG is None:
        _PROG = build_program()
    return _PROG


def kernel(trace=False, **inputs):
    from concourse.bass_utils import run_bass_kernel_spmd
    nc = get_program()
    in_maps = shard_inputs(**inputs)
    res = run_bass_kernel_spmd(nc, in_maps, core_ids=list(range(N_CORES)),
                               trace=trace)
    out = np.concatenate([np.asarray(res.results[c]["out"], dtype=np.float32)
                          for c in range(N_CORES)], axis=0)
    if trace:
        kernel.last_result = res
    return out



# revision 11
# speedup vs baseline: 1.1018x; 1.0020x over previous
"""DCell-style hierarchical NN (gather -> 3x [Linear+Tanh+BatchNorm] -> root)
on 8 Trainium2 NeuronCores.

Sharding: tree-sharding ("model parallel over subsystems") — core c owns
L1 subsystems [64c, 64c+64), L2 parents [8c, 8c+8), L3 parent c, and the
FULL batch B=4096.  All BatchNorm statistics at levels 1-3 are then
core-local (full batch present on every core); the only cross-core
communication is a single 1 MB AllReduce of the root layer's partial
products.  BatchNorm is folded at runtime into the next level's weights:
with h = tanh(W x + b) kept pre-BN, o = a*h + c where
a = g*rsqrt(var+eps), c = beta - mean*a, so the next level uses
W' = W*a (per input column) and b' = b + W@c.

Device layout: features on partitions, batch on the free axis.  Matmul
operands are bf16 (PE runs 1 cycle/row vs 4 for fp32), accumulation /
statistics / normalization all fp32.  L1 uses block-diagonal weights
(8 subsystems of 16->20 per K=128 matmul).  L2 outputs are packed four
parents per PSUM bank in 32-row slots (24 real features zero-padded to
32 — PE column tiles must be 32-aligned).  Stats use bn_stats/bn_aggr
(one DVE pass for mean+var); rsqrt is a Newton iteration on the DVE to
avoid switching the ACT table set away from Tanh.  Matmuls of the same
PE tiling mode are batched to avoid mode-switch drains.
"""

import numpy as np
import ml_dtypes

BF16 = ml_dtypes.bfloat16
N_CORES = 8
B = 4096
BT = 512
NBT = B // BT
EPS = 1e-5
MAGIC = 0x5F3759DF

_PROG = None  # cached program across calls


# ---------------------------------------------------------------- device code

def _rsqrt_newton(nc, AL, y, s, t, magic, iters=3):
    """y = rsqrt(s), all APs same shape, f32 (magic: int32)."""
    import concourse.mybir as mybir
    i32 = mybir.dt.int32
    nc.vector.tensor_scalar(out=t.bitcast(i32), in0=s.bitcast(i32),
                            scalar1=1, scalar2=None, op0=AL.arith_shift_right)
    nc.vector.tensor_tensor(out=y.bitcast(i32), in0=magic, in1=t.bitcast(i32),
                            op=AL.subtract)
    for _ in range(iters):
        nc.vector.tensor_tensor(out=t, in0=y, in1=y, op=AL.mult)
        nc.vector.tensor_tensor(out=t, in0=t, in1=s, op=AL.mult)
        nc.vector.tensor_scalar(out=t, in0=t, scalar1=-0.5, scalar2=1.5,
                                op0=AL.mult, op1=AL.add)
        nc.vector.tensor_tensor(out=y, in0=y, in1=t, op=AL.mult)


def build_program():
    import concourse.bacc as bacc
    import concourse.mybir as mybir
    import concourse.tile as tile

    f32 = mybir.dt.float32
    bf16 = mybir.dt.bfloat16
    i32 = mybir.dt.int32
    AL = mybir.AluOpType
    TANH = mybir.ActivationFunctionType.Tanh
    IDENT = mybir.ActivationFunctionType.Identity

    nc = bacc.Bacc("TRN2", target_bir_lowering=False, debug=False,
                   num_devices=N_CORES)

    # ------------------------------------------------ DRAM I/O (per core)
    xgd = nc.dram_tensor("xg", [8, 128, B], bf16, kind="ExternalInput")
    w1d = nc.dram_tensor("w1", [128, 1280], bf16, kind="ExternalInput")
    w2ad = nc.dram_tensor("w2a", [128, 256], f32, kind="ExternalInput")
    w2bd = nc.dram_tensor("w2b", [128, 256], f32, kind="ExternalInput")
    sad = nc.dram_tensor("sa", [128, 30], f32, kind="ExternalInput")
    s96d = nc.dram_tensor("s96", [128, 68], f32, kind="ExternalInput")
    s32d = nc.dram_tensor("s32", [32, 67], f32, kind="ExternalInput")
    s64d = nc.dram_tensor("s64", [64, 67], f32, kind="ExternalInput")
    b2hd = nc.dram_tensor("b2h", [128, 2], f32, kind="ExternalInput")
    outd = nc.dram_tensor("out", [B // N_CORES, 64], f32, kind="ExternalOutput")
    BS = B // N_CORES
    cc_in = nc.dram_tensor("cc_in", [64, B], bf16)
    cc_out = nc.dram_tensor("cc_out", [64, B], bf16, addr_space="Shared")

    with tile.TileContext(nc) as tc:
        sbS = tc.alloc_tile_pool(name="sbS", bufs=1)
        sbH1 = tc.alloc_tile_pool(name="sbH1", bufs=1)
        sbX = tc.alloc_tile_pool(name="sbX", bufs=1, side="right")
        psmm = tc.alloc_tile_pool(name="psmm", bufs=4, space="PSUM")

        # static tiles
        w1sb = sbS.tile([128, 1280], bf16, name="w1sb")
        w2asb = sbS.tile([128, 256], f32, name="w2asb")
        w2bsb = sbS.tile([128, 256], f32, name="w2bsb")
        sAsb = sbS.tile([128, 30], f32, name="sAsb")
        s96sb = sbS.tile([128, 68], f32, name="s96sb")
        s32sb = sbS.tile([32, 67], f32, name="s32sb")
        s64sb = sbS.tile([64, 67], f32, name="s64sb")
        b2hsb = sbS.tile([128, 2], f32, name="b2hsb")
        stA = sbS.tile([128, 384], f32, name="stA")   # 8 parents * 8bt * 6
        stB = sbS.tile([128, 96], f32, name="stB")    # 2 groups
        st2 = sbS.tile([128, 96], f32, name="st2")
        st3 = sbS.tile([32, 48], f32, name="st3")
        stR = sbS.tile([64, 48], f32, name="stR")
        aggA = sbS.tile([128, 16], f32, name="aggA")
        aggB = sbS.tile([128, 4], f32, name="aggB")
        agg2 = sbS.tile([128, 4], f32, name="agg2")
        agg3 = sbS.tile([32, 2], f32, name="agg3")
        aggR = sbS.tile([64, 2], f32, name="aggR")
        magic = sbS.tile([128, 16], i32, name="magic")
        nsS = sbS.tile([128, 16], f32, name="nsS")
        nsT = sbS.tile([128, 16], f32, name="nsT")
        nsY = sbS.tile([128, 16], f32, name="nsY")
        ctm = sbS.tile([128, 8], f32, name="ctm")
        aA = sbS.tile([128, 8], f32, name="aA")
        cA = sbS.tile([128, 8], f32, name="cA")
        aB = sbS.tile([128, 2], f32, name="aB")
        cB = sbS.tile([128, 2], f32, name="cB")
        a2 = sbS.tile([128, 2], f32, name="a2")
        c2 = sbS.tile([128, 2], f32, name="c2")
        a3 = sbS.tile([32, 1], f32, name="a3")
        c3 = sbS.tile([32, 1], f32, name="c3")
        srt = sbS.tile([64, 1], f32, name="srt")
        drt = sbS.tile([64, 1], f32, name="drt")
        w2abf = sbS.tile([128, 256], bf16, name="w2abf")
        w2bbf = sbS.tile([128, 256], bf16, name="w2bbf")
        w3abf = sbS.tile([128, 32], bf16, name="w3abf")
        w3bbf = sbS.tile([128, 32], bf16, name="w3bbf")
        wrbf = sbS.tile([32, 64], bf16, name="wrbf")
        b2p = sbS.tile([128, 2], f32, name="b2p")
        b3sb = sbS.tile([32, 1], f32, name="b3sb")
        brc = sbS.tile([64, 1], f32, name="brc")

        h1a = sbH1.tile([128, 8 * B], bf16, name="h1a")
        h1b = sbH1.tile([128, 2 * B], bf16, name="h1b")
        xsb = sbX.tile([128, 8 * B], bf16, name="xsb")

        nc.vector.memset(magic[:], MAGIC)

        # ------------------------------------------------ input DMAs
        # small params first (they gate the first activations), then the
        # bulk gathered input chunk by chunk so L1 compute overlaps the
        # remaining transfers.
        nc.sync.dma_start(sAsb[:], sad[:])
        nc.sync.dma_start(w1sb[:], w1d[:])
        nc.sync.dma_start(b2hsb[:], b2hd[:])
        nc.sync.dma_start(w2asb[:], w2ad[:])
        nc.sync.dma_start(w2bsb[:], w2bd[:])
        nc.sync.dma_start(s96sb[:], s96d[:])
        nc.sync.dma_start(s32sb[:], s32d[:])
        nc.sync.dma_start(s64sb[:], s64d[:])
        for p in range(8):
            nc.sync.dma_start(xsb[:, p * B:(p + 1) * B], xgd[p, :, :])

        # ------------------------------------------------ level 1
        # A pass: per-parent [K=128 genes] -> [128 features] (mode 128x128),
        # weight-stationary (p outer), batch-tile pairs share a 2-bank psum
        # so each ACT covers [128, 1024].  B pass (remaining 32 features x 4
        # parents packed per bank, mode 128x32) is interleaved per group so
        # its stats finish early.
        def l1_a(p):
            for b2 in range(NBT // 2):
                c0 = b2 * 2 * BT
                psA = psmm.tile([128, 2 * BT], f32, name=f"psA_{p}_{b2}",
                                tag="mm")
                for h in range(2):
                    nc.tensor.matmul(
                        psA[:, h * BT:(h + 1) * BT],
                        w1sb[:, p * 160: p * 160 + 128],
                        xsb[:, p * B + c0 + h * BT: p * B + c0 + (h + 1) * BT],
                        start=True, stop=True)
                ha = h1a[:, p * B + c0: p * B + c0 + 2 * BT]
                nc.scalar.activation(ha, psA[:], TANH, bias=sAsb[:, p:p + 1])
                for h in range(2):
                    bt = 2 * b2 + h
                    nc.vector.bn_stats(
                        stA[:, p * 48 + bt * 6: p * 48 + bt * 6 + 6],
                        h1a[:, p * B + bt * BT: p * B + (bt + 1) * BT])
            nc.vector.bn_aggr(aggA[:, 2 * p:2 * p + 2],
                              stA[:, p * 48:(p + 1) * 48])

        def l1_b(g, b2s):
            for b2 in b2s:
                c0 = b2 * 2 * BT
                psB = psmm.tile([128, 2 * BT], f32, name=f"psB_{b2}_{g}",
                                tag="mm")
                for h in range(2):
                    ch = c0 + h * BT
                    for q in range(4):
                        p = 4 * g + q
                        nc.tensor.matmul(
                            psB[32 * q:32 * q + 32, h * BT:(h + 1) * BT],
                            w1sb[:, p * 160 + 128:(p + 1) * 160],
                            xsb[:, p * B + ch: p * B + ch + BT],
                            start=True, stop=True,
                            tile_position=(0, 32 * q),
                            skip_group_check=True)
                hb = h1b[:, g * B + c0: g * B + c0 + 2 * BT]
                nc.scalar.activation(hb, psB[:], TANH,
                                     bias=sAsb[:, 24 + g:25 + g])
                for h in range(2):
                    bt = 2 * b2 + h
                    nc.vector.bn_stats(
                        stB[:, g * 48 + bt * 6: g * 48 + bt * 6 + 6],
                        h1b[:, g * B + bt * BT: g * B + (bt + 1) * BT])

        for p in range(4):
            l1_a(p)
        l1_b(0, range(0, 2))
        l1_a(4)
        l1_b(0, range(2, 4))
        nc.vector.bn_aggr(aggB[:, 0:2], stB[:, 0:48])
        for p in range(5, 8):
            l1_a(p)
        l1_b(1, range(0, 4))
        nc.vector.bn_aggr(aggB[:, 2:4], stB[:, 48:96])
        sbX.release()

        # ------------------------------------------------ L1 stats -> fold
        nc.vector.tensor_scalar(out=nsS[:, 0:8], in0=aggA[:, 1::2],
                                scalar1=EPS, scalar2=None, op0=AL.add)
        _rsqrt_newton(nc, AL, nsY[:, 0:8], nsS[:, 0:8], nsT[:, 0:8], magic[:, 0:8])
        nc.vector.tensor_tensor(out=aA[:], in0=nsY[:, 0:8], in1=sAsb[:, 8:16],
                                op=AL.mult)
        nc.vector.tensor_tensor(out=ctm[:], in0=aggA[:, 0::2], in1=aA[:], op=AL.mult)
        nc.vector.tensor_tensor(out=cA[:], in0=sAsb[:, 16:24], in1=ctm[:],
                                op=AL.subtract)
        nc.vector.tensor_scalar(out=nsS[:, 8:10], in0=aggB[:, 1::2],
                                scalar1=EPS, scalar2=None, op0=AL.add)
        _rsqrt_newton(nc, AL, nsY[:, 8:10], nsS[:, 8:10], nsT[:, 8:10], magic[:, 8:10])
        nc.vector.tensor_tensor(out=aB[:], in0=nsY[:, 8:10], in1=sAsb[:, 26:28],
                                op=AL.mult)
        nc.vector.tensor_tensor(out=ctm[:, 0:2], in0=aggB[:, 0::2], in1=aB[:],
                                op=AL.mult)
        nc.vector.tensor_tensor(out=cB[:], in0=sAsb[:, 28:30], in1=ctm[:, 0:2],
                                op=AL.subtract)
        # fold BN1 into W2 (bf16) and bias.  w2b is block-diagonal per group
        # ([128, 128] covering 4 parents), so its fold is one op per group.
        for p in range(8):
            nc.vector.tensor_scalar(out=w2abf[:, 32 * p:32 * p + 32],
                                    in0=w2asb[:, 32 * p:32 * p + 32],
                                    scalar1=aA[:, p:p + 1], scalar2=None,
                                    op0=AL.mult)
        for g in range(2):
            nc.vector.tensor_scalar(out=w2bbf[:, 128 * g:128 * g + 128],
                                    in0=w2bsb[:, 128 * g:128 * g + 128],
                                    scalar1=aB[:, g:g + 1], scalar2=None,
                                    op0=AL.mult)
        psT2 = [psmm.tile([128, BT], f32, name=f"psb2_{g}", tag="mm")
                for g in range(2)]
        for g in range(2):
            for q in range(4):
                p = 4 * g + q
                nc.tensor.matmul(psT2[g][32 * q:32 * q + 32, 0:1],
                                 w2asb[:, 32 * p:32 * p + 32], cA[:, p:p + 1],
                                 start=True, stop=False,
                                 tile_position=(0, 32 * q),
                                 skip_group_check=True)
        for g in range(2):
            nc.tensor.matmul(psT2[g][:, 0:1], w2bsb[:, 128 * g:128 * g + 128],
                             cB[:, g:g + 1], start=False, stop=True,
                             skip_group_check=True)
        for g in range(2):
            nc.scalar.activation(b2p[:, g:g + 1], psT2[g][:, 0:1], IDENT,
                                 bias=b2hsb[:, g:g + 1])

        # ------------------------------------------------ level 2
        sbH2 = tc.alloc_tile_pool(name="sbH2", bufs=1, side="right")
        h2 = sbH2.tile([128, 2 * B], bf16, name="h2")
        for b2 in range(NBT // 2):
            c0 = b2 * 2 * BT
            ps2g = [psmm.tile([128, 2 * BT], f32, name=f"ps2_{b2}_{g}",
                              tag="mm") for g in range(2)]
            # A contributions: (128x32) col-tiled mode; both batch halves
            # back-to-back so each stationary weight is loaded once
            for g in range(2):
                for q in range(4):
                    p = 4 * g + q
                    for h in range(2):
                        ch = c0 + h * BT
                        nc.tensor.matmul(
                            ps2g[g][32 * q:32 * q + 32, h * BT:(h + 1) * BT],
                            w2abf[:, 32 * p:32 * p + 32],
                            h1a[:, p * B + ch: p * B + ch + BT],
                            start=True, stop=False,
                            tile_position=(0, 32 * q),
                            skip_group_check=True)
            # B contributions: one block-diagonal matmul per (g, half)
            for g in range(2):
                for h in range(2):
                    ch = c0 + h * BT
                    nc.tensor.matmul(
                        ps2g[g][:, h * BT:(h + 1) * BT],
                        w2bbf[:, 128 * g:128 * g + 128],
                        h1b[:, g * B + ch: g * B + ch + BT],
                        start=False, stop=True,
                        skip_group_check=True)
            for g in range(2):
                h2s = h2[:, g * B + c0: g * B + c0 + 2 * BT]
                nc.scalar.activation(h2s, ps2g[g][:], TANH, bias=b2p[:, g:g + 1])
                for h in range(2):
                    bt = 2 * b2 + h
                    nc.vector.bn_stats(
                        st2[:, g * 48 + bt * 6: g * 48 + bt * 6 + 6],
                        h2[:, g * B + bt * BT: g * B + (bt + 1) * BT])
        sbH1.release()

        # ------------------------------------------------ L2 stats -> fold
        for g in range(2):
            nc.vector.bn_aggr(agg2[:, 2 * g:2 * g + 2], st2[:, g * 48:(g + 1) * 48])
        nc.vector.tensor_scalar(out=nsS[:, 10:12], in0=agg2[:, 1::2],
                                scalar1=EPS, scalar2=None, op0=AL.add)
        _rsqrt_newton(nc, AL, nsY[:, 10:12], nsS[:, 10:12],
                      nsT[:, 10:12], magic[:, 10:12])
        nc.vector.tensor_tensor(out=a2[:], in0=nsY[:, 10:12],
                                in1=s96sb[:, 64:66], op=AL.mult)
        nc.vector.tensor_tensor(out=ctm[:, 2:4], in0=agg2[:, 0::2], in1=a2[:],
                                op=AL.mult)
        nc.vector.tensor_tensor(out=c2[:], in0=s96sb[:, 66:68],
                                in1=ctm[:, 2:4], op=AL.subtract)
        nc.vector.tensor_scalar(out=w3abf[:], in0=s96sb[:, 0:32],
                                scalar1=a2[:, 0:1], scalar2=None, op0=AL.mult)
        nc.vector.tensor_scalar(out=w3bbf[:], in0=s96sb[:, 32:64],
                                scalar1=a2[:, 1:2], scalar2=None, op0=AL.mult)
        psT3 = psmm.tile([32, 1], f32, name="psT3", tag="mm")
        nc.tensor.matmul(psT3[:], s96sb[:, 0:32], c2[:, 0:1], start=True, stop=False)
        nc.tensor.matmul(psT3[:], s96sb[:, 32:64], c2[:, 1:2], start=False, stop=True)
        nc.scalar.activation(b3sb[:], psT3[:], IDENT, bias=s32sb[:, 64:65])

        # ------------------------------------------------ level 3
        sbH3 = tc.alloc_tile_pool(name="sbH3", bufs=1)
        h3 = sbH3.tile([32, B], bf16, name="h3")
        for b2 in range(NBT // 2):
            c0 = b2 * 2 * BT
            ps3 = psmm.tile([32, 2 * BT], f32, name=f"ps3_{b2}", tag="mm")
            for h in range(2):
                nc.tensor.matmul(ps3[:, h * BT:(h + 1) * BT], w3abf[:],
                                 h2[:, c0 + h * BT: c0 + (h + 1) * BT],
                                 start=True, stop=False)
            for h in range(2):
                nc.tensor.matmul(ps3[:, h * BT:(h + 1) * BT], w3bbf[:],
                                 h2[:, B + c0 + h * BT: B + c0 + (h + 1) * BT],
                                 start=False, stop=True)
            h3s = h3[:, c0:c0 + 2 * BT]
            nc.scalar.activation(h3s, ps3[:], TANH, bias=b3sb[:])
            for h in range(2):
                bt = 2 * b2 + h
                nc.vector.bn_stats(st3[:, bt * 6: bt * 6 + 6],
                                   h3[:, bt * BT:(bt + 1) * BT])
        sbH2.release()

        nc.vector.bn_aggr(agg3[:], st3[:])
        nc.vector.tensor_scalar(out=nsS[0:32, 12:13], in0=agg3[:, 1:2],
                                scalar1=EPS, scalar2=None, op0=AL.add)
        _rsqrt_newton(nc, AL, nsY[0:32, 12:13], nsS[0:32, 12:13],
                      nsT[0:32, 12:13], magic[0:32, 12:13])
        nc.vector.tensor_tensor(out=a3[:], in0=nsY[0:32, 12:13],
                                in1=s32sb[:, 65:66], op=AL.mult)
        nc.vector.tensor_tensor(out=ctm[0:32, 4:5], in0=agg3[:, 0:1], in1=a3[:],
                                op=AL.mult)
        nc.vector.tensor_tensor(out=c3[:], in0=s32sb[:, 66:67],
                                in1=ctm[0:32, 4:5], op=AL.subtract)
        nc.vector.tensor_scalar(out=wrbf[:], in0=s32sb[:, 0:64], scalar1=a3[:],
                                scalar2=None, op0=AL.mult)
        psT4 = psmm.tile([64, 1], f32, name="psT4", tag="mm")
        nc.tensor.matmul(psT4[:], s32sb[:, 0:64], c3[:], start=True, stop=True)
        nc.scalar.copy(brc[:], psT4[:])

        # ------------- root partial + bf16 AllReduce + split-partition tail
        # Single collective (this runtime hangs on a second one): AllReduce
        # the partial products, then compute the global stats locally with
        # the full batch folded onto 128 partitions (one pass over [128,
        # B/2] instead of [64, B]), and normalize only the own batch slice.
        sbT = tc.alloc_tile_pool(name="sbT", bufs=1, side="right")
        partial = sbT.tile([64, B], bf16, name="partial")
        for b2 in range(NBT // 2):
            c0 = b2 * 2 * BT
            psr = psmm.tile([64, 2 * BT], f32, name=f"psr_{b2}", tag="mm")
            for h in range(2):
                nc.tensor.matmul(psr[:, h * BT:(h + 1) * BT], wrbf[:],
                                 h3[:, c0 + h * BT: c0 + (h + 1) * BT],
                                 start=True, stop=True)
            nc.scalar.activation(partial[:, c0:c0 + 2 * BT], psr[:], IDENT,
                                 bias=brc[:])
            # ship each chunk to the collective buffer as it is produced
            nc.sync.dma_start(cc_in[:, c0:c0 + 2 * BT],
                              partial[:, c0:c0 + 2 * BT])
        red2 = sbT.tile([128, B // 2], bf16, name="red2")
        h2r = sbT.tile([128, B // 2], bf16, name="h2r")
        sq2 = sbT.tile([128, B // 2], bf16, name="sq2")
        stG = sbT.tile([128, 2], f32, name="stG")
        stH = sbT.tile([64, 2], f32, name="stH")
        agR2 = sbT.tile([64, 2], f32, name="agR2")
        brD = sbT.tile([128, 1], f32, name="brD")
        redo = sbT.tile([64, BS], bf16, name="redo")
        hro = sbT.tile([64, BS], f32, name="hro")
        outTc = sbT.tile([64, BS], f32, name="outTc")
        outSc = sbT.tile([128, BS // 2], f32, name="outSc")
        nc.vector.tensor_copy(brD[0:64, :], s64sb[:, 0:1])
        nc.sync.dma_start(brD[64:128, :], s64sb[:, 0:1])
        nc.gpsimd.collective_compute(
            "AllReduce", AL.add,
            replica_groups=[list(range(N_CORES))],
            ins=[cc_in[:].opt()], outs=[cc_out[:].opt()])
        # [64, B] -> [128, B/2]: batch half h on partitions 64h..64h+64
        nc.sync.dma_start(red2[0:64, :], cc_out[:, 0:B // 2])
        nc.sync.dma_start(red2[64:128, :], cc_out[:, B // 2:B])
        import concourse.bass as bass_mod
        pid = nc.sync.partition_id()
        off = pid * BS
        nc.sync.dma_start(redo[:], cc_out[:, bass_mod.ds(off, BS)])
        # global stats: one tanh+accum pass and one square pass over the
        # half-stacked layout, then fold the upper 64 partitions down.
        nc.scalar.activation(h2r[:], red2[:], TANH, bias=brD[:],
                             accum_out=stG[:, 0:1])
        nc.vector.tensor_tensor(out=sq2[:], in0=h2r[:], in1=h2r[:], op=AL.mult)
        nc.vector.tensor_reduce(out=stG[:, 1:2], in_=sq2[:],
                                axis=mybir.AxisListType.X, op=AL.add)
        nc.sync.dma_start(stH[:], stG[64:128, :])
        nc.vector.tensor_tensor(out=agR2[:], in0=stG[0:64, :], in1=stH[:],
                                op=AL.add)
        # mean = Sh/B ; var = Sh2/B - mean^2
        nc.vector.tensor_scalar(out=aggR[:, 0:1], in0=agR2[:, 0:1],
                                scalar1=1.0 / B, scalar2=None, op0=AL.mult)
        nc.vector.tensor_scalar(out=nsT[0:64, 14:15], in0=aggR[:, 0:1],
                                scalar1=aggR[:, 0:1], scalar2=None, op0=AL.mult)
        nc.vector.tensor_scalar(out=aggR[:, 1:2], in0=agR2[:, 1:2],
                                scalar1=1.0 / B, scalar2=nsT[0:64, 14:15],
                                op0=AL.mult, op1=AL.subtract)
        nc.vector.tensor_scalar(out=nsS[0:64, 13:14], in0=aggR[:, 1:2],
                                scalar1=EPS, scalar2=None, op0=AL.add)
        _rsqrt_newton(nc, AL, nsY[0:64, 13:14], nsS[0:64, 13:14],
                      nsT[0:64, 13:14], magic[0:64, 13:14])
        nc.vector.tensor_tensor(out=srt[:], in0=nsY[0:64, 13:14],
                                in1=s64sb[:, 1:2], op=AL.mult)
        nc.vector.tensor_tensor(out=ctm[0:64, 5:6], in0=aggR[:, 0:1], in1=srt[:],
                                op=AL.mult)
        nc.vector.tensor_tensor(out=drt[:], in0=s64sb[:, 2:3],
                                in1=ctm[0:64, 5:6], op=AL.subtract)
        # normalize the local batch slice only
        nc.scalar.activation(hro[:], redo[:], TANH, bias=s64sb[:, 0:1])
        nc.vector.tensor_scalar(out=outTc[:], in0=hro[:],
                                scalar1=srt[:], scalar2=drt[:],
                                op0=AL.mult, op1=AL.add)
        # transpose [64, 512] -> [512, 64] via PE, 128 batch rows at a time
        for t in range(BS // 128):
            pstr = psmm.tile([128, 64], f32, name=f"pstr_{t}", tag="mm")
            nc.tensor.transpose(pstr[:], outTc[:, t * 128:(t + 1) * 128],
                                s64sb[:, 3:67])
            nc.vector.tensor_copy(outSc[:, t * 64:(t + 1) * 64], pstr[:])
        nc.sync.dma_start(outd[:].rearrange("(t p) o -> p t o", p=128),
                          outSc[:].rearrange("p (t o) -> p t o", o=64))

        sbT.release()
        sbH3.release()
        sbS.release()
        psmm.release()

    nc.compile()
    return nc


# ---------------------------------------------------------------- host side

def shard_inputs(mutant_state, gene_idx, W1, b1, g1, beta1, W2, b2, g2, beta2,
                 W3, b3, g3, beta3, Wr, br, gr, betar):
    """Build the per-core in_maps."""
    mutant_state = np.asarray(mutant_state, dtype=np.float32)
    gene_idx = np.asarray(gene_idx)
    W1 = np.asarray(W1, np.float32); b1 = np.asarray(b1, np.float32)
    g1 = np.asarray(g1, np.float32); beta1 = np.asarray(beta1, np.float32)
    W2 = np.asarray(W2, np.float32); b2 = np.asarray(b2, np.float32)
    g2 = np.asarray(g2, np.float32); beta2 = np.asarray(beta2, np.float32)
    W3 = np.asarray(W3, np.float32); b3 = np.asarray(b3, np.float32)
    g3 = np.asarray(g3, np.float32); beta3 = np.asarray(beta3, np.float32)
    Wr = np.asarray(Wr, np.float32); br = np.asarray(br, np.float32)
    gr = np.asarray(gr, np.float32); betar = np.asarray(betar, np.float32)

    MT = np.ascontiguousarray(mutant_state.astype(BF16).T)  # [N, B] bf16
    eye = np.eye(64, dtype=np.float32)

    in_maps = []
    for c in range(N_CORES):
        idx = gene_idx[64 * c:64 * (c + 1)].reshape(8, 128)
        xg = np.ascontiguousarray(MT[idx])                 # [8, 128, B] bf16

        W1c = W1[64 * c:64 * (c + 1)].reshape(8, 8, 20, 16)
        blk = np.zeros((8, 128, 160), np.float32)
        for sl in range(8):
            blk[:, 16 * sl:16 * (sl + 1), 20 * sl:20 * (sl + 1)] = \
                W1c[:, sl].transpose(0, 2, 1)
        w1 = np.ascontiguousarray(
            blk.transpose(1, 0, 2).reshape(128, 1280)).astype(BF16)

        def sAcols(v):  # per-subsystem vec [64, 20] -> A [128,8], Bpack [128,2]
            vb = v[64 * c:64 * (c + 1)].reshape(8, 160)
            A = np.ascontiguousarray(vb[:, :128].T)
            Bp = np.ascontiguousarray(
                vb[:, 128:].reshape(2, 4, 32).transpose(1, 2, 0).reshape(128, 2))
            return A, Bp

        b1A, b1B = sAcols(b1); g1A, g1B = sAcols(g1); be1A, be1B = sAcols(beta1)
        sa = np.concatenate([b1A, g1A, be1A, b1B, g1B, be1B], axis=1)  # [128,30]

        # W2: lhsT layouts, 24 out-features zero-padded to 32-row slots
        W2c = W2[8 * c:8 * (c + 1)]                                    # [8,24,160]
        w2a = np.zeros((128, 8, 32), np.float32)
        w2a[:, :, :24] = W2c[:, :, :128].transpose(2, 0, 1)            # [128,8,24]
        w2a = w2a.reshape(128, 256)
        # block-diagonal per group: rows 32q+r = parent 4g+q's tail input
        # feature 128+r; cols 32q+o = that parent's (padded) output feature o
        w2b = np.zeros((128, 2, 128), np.float32)
        for g in range(2):
            for q in range(4):
                w2b[32 * q:32 * q + 32, g, 32 * q:32 * q + 24] = \
                    W2c[4 * g + q, :, 128:].T
        w2b = np.ascontiguousarray(
            np.concatenate([w2b[:, 0, :], w2b[:, 1, :]], axis=1))

        def pack128(v):  # [8, 24] per-parent -> [128, 2] padded 32-slots
            out = np.zeros((2, 4, 32), np.float32)
            out[:, :, :24] = v[8 * c:8 * (c + 1)].reshape(2, 4, 24)
            return np.ascontiguousarray(out.transpose(1, 2, 0).reshape(128, 2))

        b2h = pack128(b2)
        # W3: input features padded 24->32 per L2 parent: [192,32] -> [256,32]
        W3T = W3[c].T                                                  # [192, 32]
        W3pad = np.zeros((8, 32, 32), np.float32)
        W3pad[:, :24, :] = W3T.reshape(8, 24, 32)
        W3pad = W3pad.reshape(256, 32)
        s96 = np.concatenate([W3pad[:128], W3pad[128:], pack128(g2),
                              pack128(beta2)], axis=1)                 # [128, 68]
        s32 = np.concatenate([np.ascontiguousarray(Wr[:, 32 * c:32 * (c + 1)].T),
                              b3[c][:, None], g3[c][:, None], beta3[c][:, None]],
                             axis=1)                                   # [32, 67]
        s64 = np.concatenate([br[:, None], gr[:, None], betar[:, None], eye],
                             axis=1)                                   # [64, 67]

        in_maps.append({
            "xg": xg,
            "w1": w1,
            "w2a": np.ascontiguousarray(w2a),
            "w2b": np.ascontiguousarray(w2b),
            "sa": np.ascontiguousarray(sa),
            "s96": np.ascontiguousarray(s96),
            "s32": np.ascontiguousarray(s32),
            "s64": np.ascontiguousarray(s64),
            "b2h": b2h,
        })
    return in_maps


def get_program():
    global _PROG
    if _PROG is None:
        _PROG = build_program()
    return _PROG


def kernel(trace=False, **inputs):
    from concourse.bass_utils import run_bass_kernel_spmd
    nc = get_program()
    in_maps = shard_inputs(**inputs)
    res = run_bass_kernel_spmd(nc, in_maps, core_ids=list(range(N_CORES)),
                               trace=trace)
    out = np.concatenate([np.asarray(res.results[c]["out"], dtype=np.float32)
                          for c in range(N_CORES)], axis=0)
    if trace:
        kernel.last_result = res
    return out

